# revision 20
# baseline (speedup 1.0000x reference)
"""Trainium2 Bass kernel for a GRU decoder with Luong attention.

Problem (hardcoded shapes): B=32, S=64, T=64, H=512, V=32000.
  out = log_softmax(decoder(inputs)) with shape [B, T, V] fp32.

Sharding: data-parallel over batch. Each of the 8 cores processes 4 batch
rows end-to-end. No collectives.

Layout: recurrence state is kept transposed, hT [128, 16] with
column = q*4 + b meaning h[b, q*128 + p] for partition p. Gate pre-acts
are computed as ghT = W_hh^T-block @ hT (weight-stationary, tiny moving
size), with the precomputed gxT folded into PSUM via an identity matmul.
Gates use tanh only (sigmoid(x) = 0.5 + 0.5*tanh(x/2)) so the whole body
runs on one activation table (tanh+exp).

Output phase: logits for rows t=0..31 (m0) are computed interleaved with
recurrence steps t=32..63; m1 afterwards. Logits are stashed in SBUF as
bf16, exp+accum gives the per-row sum, log-softmax subtract is a
tensor_scalar / biased-Identity, output tensor is bf16 (host casts).

Per-core row order for the 256 output rows is t-major: r = t*4 + b_local.
"""

from contextlib import ExitStack

import numpy as np
import ml_dtypes

import concourse.bacc as bacc
import concourse.bass as bass
import concourse.mybir as mybir
import concourse.tile as tile
from concourse.masks import make_identity

F32 = mybir.dt.float32
BF16 = mybir.dt.bfloat16
I32 = mybir.dt.int32
AF = mybir.ActivationFunctionType
ALU = mybir.AluOpType
AX = mybir.AxisListType
F32R = mybir.dt.float32r


def rr(ap):
    return ap.bitcast(F32R)


B, S, T, H, V = 32, 64, 64, 512, 32000
NC = 8
BL = B // NC          # 4 local batch rows
R = T * BL            # 256 local output rows, r = t*BL + b
VCHUNK = 500          # vocab chunk for the output matmul
NVCH = V // VCHUNK    # 64 chunks per m-block
NJ2 = NVCH // 2       # 32 wo-tile groups per m-block ([128,1000] tiles)
EGRP = 2000           # exp group width
NEGR = V // EGRP      # 16 exp groups per m-block
L2W = 8000            # lts2 width (m1 chunks j < 16 land here)
NEG = -1e30
OC = 2000             # drain/store chunk


def build_program():
    nc = bacc.Bacc(None, target_bir_lowering=False, debug=False)

    # ---- DRAM parameters (per-core slices prepared on host) ----
    emb_d = nc.declare_dram_parameter("emb", [V, H], F32, isOutput=False)
    ids_d = nc.declare_dram_parameter("ids", [2, 128, 1], I32, isOutput=False)
    h0T_d = nc.declare_dram_parameter("h0T", [128, 16], F32, isOutput=False)
    encT_d = nc.declare_dram_parameter("encT", [H, BL * S], BF16, isOutput=False)
    encS_d = nc.declare_dram_parameter("encS", [S, BL * H], BF16, isOutput=False)
    maskb_d = nc.declare_dram_parameter("maskb", [1, BL * S], BF16, isOutput=False)
    actmT_d = nc.declare_dram_parameter("actmT", [128, T * 16], F32, isOutput=False)
    wihT_d = nc.declare_dram_parameter("wihT", [H, 3 * H], BF16, isOutput=False)
    whhT_d = nc.declare_dram_parameter("whhT", [H, 3 * H], F32, isOutput=False)
    bgx_d = nc.declare_dram_parameter("bgx", [1, 3 * H], F32, isOutput=False)
    bhn2_d = nc.declare_dram_parameter("bhn2", [1, H], F32, isOutput=False)
    wccT_d = nc.declare_dram_parameter("wccT", [2 * H, H], BF16, isOutput=False)
    bcc_d = nc.declare_dram_parameter("bcc", [128, 4], F32, isOutput=False)
    woT_d = nc.declare_dram_parameter("woT", [H, V], BF16, isOutput=False)
    ones_d = nc.declare_dram_parameter("onesd", [1, 256], F32, isOutput=False)
    out_d = nc.declare_dram_parameter("out", [R, V], BF16, isOutput=True)

    with tile.TileContext(nc) as tc, ExitStack() as stk:
        constp = stk.enter_context(tc.tile_pool(name="const", bufs=1))
        histp = stk.enter_context(tc.tile_pool(name="hist", bufs=1))
        hotp = stk.enter_context(tc.tile_pool(name="hot", bufs=1))
        ltsp = stk.enter_context(tc.tile_pool(name="lts", bufs=1))
        woPp = stk.enter_context(tc.tile_pool(name="woP", bufs=3))
        woSp = stk.enter_context(tc.tile_pool(name="woS", bufs=3))
        edp = stk.enter_context(tc.tile_pool(name="edump", bufs=1))
        sump = stk.enter_context(tc.tile_pool(name="sums", bufs=6))
        osp = stk.enter_context(tc.tile_pool(name="ost", bufs=2))
        hTp = stk.enter_context(tc.tile_pool(name="hT", bufs=3))
        gp = stk.enter_context(tc.tile_pool(name="gates", bufs=3))
        attp = stk.enter_context(tc.tile_pool(name="att", bufs=2))
        ps_g = stk.enter_context(tc.tile_pool(name="ps_g", bufs=3, space="PSUM"))
        ps_a = stk.enter_context(tc.tile_pool(name="ps_a", bufs=2, space="PSUM"))
        ps_o = stk.enter_context(tc.tile_pool(name="ps_o", bufs=3, space="PSUM"))

        # ---- constants / weights ----
        ident = constp.tile([128, 128], F32, tag="ident")
        make_identity(nc, ident[:])

        ones_f = constp.tile([1, 256], F32, tag="ones_f")
        nc.sync.dma_start(rr(ones_f[:]), rr(ones_d[:]))
        ones_b = constp.tile([1, 128], BF16, tag="ones_b")
        nc.vector.memset(ones_b[:], 1.0)

        half = constp.tile([128, 1], F32, tag="half")
        nc.vector.memset(half[:], 0.5)
        h0T = constp.tile([128, 16], F32, tag="h0T")
        nc.sync.dma_start(rr(h0T[:]), rr(h0T_d[:]))
        identr = constp.tile([128, 128], F32, tag="identr")
        bhn2 = constp.tile([1, H], F32, tag="bhn2")
        nc.sync.dma_start(rr(bhn2[:]), rr(bhn2_d[:]))
        whh = []
        for q in range(4):
            ht = constp.tile([128, 3 * H], F32, tag=f"whh{q}")
            eng = nc.scalar if q % 2 == 0 else nc.sync
            eng.dma_start(rr(ht[:]), rr(whhT_d[q * 128:(q + 1) * 128, :]))
            whh.append(ht)

        # embedding gather first on the Pool queue (P1 critical path)
        gat = []
        for m in range(2):
            ids_t = constp.tile([128, 1], I32, tag=f"ids{m}", name=f"ids{m}")
            nc.gpsimd.dma_start(ids_t[:], ids_d[m])
            xs_t = constp.tile([128, H], F32, tag=f"xs{m}", name=f"xs{m}")
            nc.gpsimd.indirect_dma_start(
                out=xs_t[:],
                out_offset=None,
                in_=emb_d[:],
                in_offset=bass.IndirectOffsetOnAxis(ap=ids_t[:, 0:1], axis=0),
            )
            gat.append(xs_t)
        maskb = constp.tile([1, BL * S], BF16, tag="maskb")
        nc.gpsimd.dma_start(maskb[:], maskb_d[:])
        actmT = constp.tile([128, T * 16], F32, tag="actmT")
        nc.gpsimd.dma_start(actmT[:], actmT_d[:])
        bcc = constp.tile([128, 4], F32, tag="bcc")
        nc.gpsimd.dma_start(bcc[:], bcc_d[:])
        encT = []
        for q in range(4):
            e = constp.tile([128, BL * S], BF16, tag=f"encT{q}")
            nc.gpsimd.dma_start(e[:], encT_d[q * 128:(q + 1) * 128, :])
            encT.append(e)
        encS = constp.tile([S, BL * H], BF16, tag="encS")
        nc.gpsimd.dma_start(encS[:], encS_d[:])
        wcc = []
        for kt in range(8):
            w = constp.tile([128, H], BF16, tag=f"wcc{kt}")
            nc.gpsimd.dma_start(w[:], wccT_d[kt * 128:(kt + 1) * 128, :])
            wcc.append(w)

        # history buffers: col = t*16 + q*4 + b
        nc.vector.tensor_copy(rr(identr[:]), ident[:])
        hnewT = histp.tile([128, T * 16], BF16, tag="hnewT")
        ctxT = histp.tile([128, T * 16], BF16, tag="ctxT")
        gxT = [histp.tile([128, T * 16], F32, tag=f"gxT{g}", name=f"gxT{g}")
           for g in range(3)]
        hot = [hotp.tile([128, R], BF16, tag=f"hot{q}", name=f"hot{q}")
           for q in range(4)]
        lts = ltsp.tile([128, V], BF16, tag="lts")
        lts2 = ltsp.tile([128, L2W], BF16, tag="lts2")
        sets = [sump.tile([128, NEGR], F32, tag=f"se{m}", name=f"sums{m}")
                for m in range(2)]

        def lts_ap(m, c0, c1):
            if m == 1 and c1 <= L2W:
                return lts2[:, c0:c1]
            return lts[:, c0:c1]

        # ---- P1: embedding gather + gxT = (xs @ W_ih.T + bias)^T ----
        with tc.tile_pool(name="p1", bufs=1) as p1p, \
             tc.tile_pool(name="p1x", bufs=2) as p1xp:
            bgx = p1p.tile([1, 3 * H], F32, tag="bgx")
            nc.sync.dma_start(rr(bgx[:]), rr(bgx_d[:]))
            wih = []
            for q in range(4):
                wt = p1p.tile([128, 3 * H], BF16, tag=f"wih{q}")
                nc.sync.dma_start(wt[:], wihT_d[q * 128:(q + 1) * 128, :])
                wih.append(wt)
            xsT = []
            for q in range(4):
                xt = p1p.tile([128, 256], BF16, tag=f"xsT{q}")
                xsT.append(xt)
            for m in range(2):
                xs_t = gat[m]
                for q in range(4):
                    tp = ps_a.tile([128, 128], F32, tag="A", name=f"tp{m}_{q}")
                    nc.tensor.transpose(tp[:], xs_t[:, q * 128:(q + 1) * 128],
                                        ident[:])
                    nc.vector.tensor_copy(xsT[q][:, m * 128:(m + 1) * 128],
                                          tp[:])
            for g in range(3):
                for qo in range(4):
                    c = g * 512 + qo * 128
                    ps = ps_a.tile([128, 256], F32, tag="A", name=f"gx{g}_{qo}")
                    for qi in range(4):
                        nc.tensor.matmul(
                            ps[:], wih[qi][:, c:c + 128], xsT[qi][:],
                            start=(qi == 0), stop=False,
                        )
                    nc.tensor.matmul(
                        ps[:], rr(bgx[0:1, c:c + 128]), rr(ones_f[0:1, :]),
                        start=False, stop=True,
                    )
                    dst = rr(gxT[g][:]).rearrange("p (t s) -> p t s", s=16)[
                        :, :, qo * 4:(qo + 1) * 4]
                    src = ps[:].rearrange("p (t b) -> p t b", b=4)
                    nc.vector.tensor_copy(dst, src)

        # ---- W_out prefetch tiles: [128, 1000] bf16, keyed (m, j2, q) ----
        # Pool queue carries q in {0,1}; SP queue carries q in {2,3}.
        wo_tiles = {}

        def emit_wo(m, j2):
            for q in range(4):
                pool = woPp if q < 2 else woSp
                eng = nc.gpsimd if q < 2 else nc.sync
                w = pool.tile([128, 1000], BF16, tag=f"wo{q}",
                              name=f"wo{m}_{j2}_{q}")
                eng.dma_start(
                    w[:], woT_d[q * 128:(q + 1) * 128,
                                j2 * 1000:(j2 + 1) * 1000])
                wo_tiles[(m, j2, q)] = w

        # ---- P5 chunk: logits for rows m*128..+128, vocab j*500..+500 ----
        def p5_chunk(m, j, cp_eng):
            j2, half = j // 2, j % 2
            hs = slice(half * VCHUNK, (half + 1) * VCHUNK)
            ps = ps_o.tile([128, VCHUNK], F32, tag="O", name=f"po{m}_{j}")
            for q in range(4):
                wt = wo_tiles[(m, j2, q)]
                if half == 1 and q == 3:
                    del wo_tiles[(m, j2, q)]
                nc.tensor.matmul(
                    ps[:], hot[q][:, m * 128:(m + 1) * 128], wt[:, hs],
                    start=(q == 0), stop=(q == 3),
                )
            dst = lts_ap(m, j * VCHUNK, (j + 1) * VCHUNK)
            if cp_eng == "v":
                nc.vector.tensor_copy(dst, ps[:])
            else:
                nc.scalar.activation(dst, ps[:], AF.Identity)

        # ---- P5 exp group: sum of exp over a 2000-wide slice ----
        def p5_exp(m, g):
            dump = edp.tile([128, EGRP], BF16, tag="edump", name=f"ed{m}_{g}")
            nc.scalar.activation(
                dump[:], lts_ap(m, g * EGRP, (g + 1) * EGRP), AF.Exp,
                accum_out=sets[m][:, g:g + 1],
            )

        # ---- P5 drain: lse + subtract + store (descending if asked) ----
        def p5_lse(m):
            stot = sump.tile([128, 1], F32, tag="stot", name=f"st{m}")
            scr = sump.tile([128, NEGR], F32, tag="scr", name=f"scr{m}")
            nc.scalar.activation(scr[:], sets[m][:], AF.Identity,
                                 accum_out=stot[:, 0:1])
            lse = sump.tile([128, 1], F32, tag="lse", name=f"ls{m}")
            nc.scalar.activation(lse[:], stot[:], AF.Ln)
            nlse = sump.tile([128, 1], F32, tag="nlse", name=f"nls{m}")
            nc.vector.tensor_scalar_mul(nlse[:], lse[:], -1.0)
            return lse, nlse

        def p5_drain_chunk(m, g, lse, nlse):
            ost = osp.tile([128, OC], BF16, tag="ost", name=f"os{m}_{g}")
            src = lts_ap(m, g * OC, (g + 1) * OC)
            if g % 4 < 3:
                nc.vector.tensor_scalar_sub(ost[:], src, lse[:, 0:1])
            else:
                nc.scalar.activation(ost[:], src, AF.Identity,
                                     bias=nlse[:, 0:1])
            nc.sync.dma_start(
                out_d[m * 128:(m + 1) * 128, g * OC:(g + 1) * OC], ost[:])

        # ---- attention + hot for a 16-step block ----
        def attn_block(blk):
            c0, c1 = blk * 256, (blk + 1) * 256
            for b in range(BL):
                sc = ps_a.tile([16, S], F32, tag="A", name=f"sc{blk}_{b}")
                for q in range(4):
                    nc.tensor.matmul(
                        sc[:], hnewT[:, c0 + q * 4 + b:c1:16],
                        encT[q][:, b * S:(b + 1) * S],
                        start=(q == 0), stop=False,
                    )
                nc.tensor.matmul(
                    sc[:], ones_b[0:1, 0:16], maskb[0:1, b * S:(b + 1) * S],
                    start=False, stop=True,
                )
                se = attp.tile([16, 1], F32, tag="se", name=f"se{blk}_{b}")
                al = attp.tile([16, S], F32, tag="al", name=f"al{blk}_{b}")
                nc.scalar.activation(al[:], sc[:], AF.Exp,
                                     accum_out=se[:, 0:1])
                rec = attp.tile([16, 1], F32, tag="rec", name=f"rc{blk}_{b}")
                nc.vector.reciprocal(rec[:], se[:])
                aln = attp.tile([16, S], F32, tag="aln", name=f"an{blk}_{b}")
                nc.vector.tensor_scalar_mul(aln[:], al[:], rec[:, 0:1])
                alT_ps = ps_a.tile([S, 16], F32, tag="A", name=f"tp{blk}_{b}")
                nc.tensor.transpose(alT_ps[:], aln[:], ident[0:16, 0:16])
                alT = attp.tile([S, 16], BF16, tag="alT", name=f"at{blk}_{b}")
                nc.vector.tensor_copy(alT[:], alT_ps[:])
                for q in range(4):
                    cx = ps_a.tile([128, 16], F32, tag="A",
                                   name=f"cx{blk}_{b}_{q}")
                    nc.tensor.matmul(
                        cx[:],
                        encS[0:S, b * H + q * 128:b * H + (q + 1) * 128],
                        alT[:],
                        start=True, stop=True,
                    )
                    nc.vector.tensor_copy(ctxT[:, c0 + q * 4 + b:c1:16], cx[:])
            for mh in range(4):
                hps = ps_a.tile([128, 64], F32, tag="A", name=f"hp{blk}_{mh}")
                for kt in range(8):
                    srcT = ctxT if kt < 4 else hnewT
                    q = kt % 4
                    rhs = srcT[:].rearrange("p (t x) -> p t x", x=16)[
                        :, blk * 16:(blk + 1) * 16, q * 4:(q + 1) * 4]
                    nc.tensor.matmul(
                        hps[:], wcc[kt][:, mh * 128:(mh + 1) * 128], rhs,
                        start=(kt == 0), stop=(kt == 7),
                    )
                nc.scalar.activation(
                    hot[mh][:, blk * 64:(blk + 1) * 64], hps[:],
                    AF.Tanh, bias=bcc[:, mh:mh + 1],
                )

        # ---- P2: GRU recurrence over T steps, P5-m0 woven in ----
        hT = h0T
        for j2 in range(3):
            emit_wo(0, j2)
        wo_next = [3, 0]
        nchunk = [0, 0]

        for t in range(T):
            cs = slice(t * 16, (t + 1) * 16)
            psums = [None, None, None]
            for g in (0, 1, 2):
                pg = ps_g.tile([128, 16], F32, tag="G", name=f"pg{t}_{g}")
                for qo in range(4):
                    co = slice(qo * 4, (qo + 1) * 4)
                    if g < 2:
                        nc.tensor.matmul(
                            pg[:, co], rr(identr[:]),
                            rr(gxT[g][:, t * 16 + qo * 4:t * 16 + qo * 4 + 4]),
                            start=True, stop=False,
                        )
                    else:
                        nc.tensor.matmul(
                            pg[:, co],
                            rr(bhn2[0:1, qo * 128:(qo + 1) * 128]),
                            rr(ones_f[0:1, 0:4]),
                            start=True, stop=False,
                        )
                    c = g * 512 + qo * 128
                    for qi in range(4):
                        nc.tensor.matmul(
                            pg[:, co],
                            rr(whh[qi][:, c:c + 128]),
                            rr(hT[:, qi * 4:(qi + 1) * 4]),
                            start=False, stop=(qi == 3),
                        )
                psums[g] = pg
            pr, pz, pn = psums
            # r = 0.5*(1+tanh(0.5*pre)) ; u = 1-z = 0.5*(1+tanh(-0.5*pre))
            th_r = gp.tile([128, 16], F32, tag="th_r", name=f"thr{t}")
            nc.scalar.activation(th_r[:], pr[:], AF.Tanh, scale=0.5)
            th_u = gp.tile([128, 16], F32, tag="th_u", name=f"thu{t}")
            nc.scalar.activation(th_u[:], pz[:], AF.Tanh, scale=-0.5)
            # n path: pn = 0.5*(gh_n + b_hn);  r*(gh_n + b_hn) = (1+th_r)*pn
            u = gp.tile([128, 16], F32, tag="u", name=f"u{t}")
            nc.scalar.activation(u[:], th_u[:], AF.Identity,
                                 bias=half[:, 0:1], scale=0.5)
            t1 = gp.tile([128, 16], F32, tag="t1", name=f"t1_{t}")
            nc.vector.tensor_tensor(t1[:], th_r[:], pn[:], ALU.mult)
            t2 = gp.tile([128, 16], F32, tag="t2", name=f"t2_{t}")
            nc.vector.tensor_tensor(t2[:], pn[:], gxT[2][:, cs], ALU.add)
            npre = gp.tile([128, 16], F32, tag="npre", name=f"np{t}")
            nc.vector.tensor_tensor(npre[:], t1[:], t2[:], ALU.add)
            nT = gp.tile([128, 16], F32, tag="nT", name=f"nT{t}")
            nc.scalar.activation(nT[:], npre[:], AF.Tanh)
            d = gp.tile([128, 16], F32, tag="d", name=f"d{t}")
            nc.vector.tensor_tensor(d[:], nT[:], hT[:], ALU.subtract)
            if t < T - 1:
                ua = gp.tile([128, 16], F32, tag="ua", name=f"ua{t}")
                nc.vector.tensor_tensor(ua[:], u[:], actmT[:, cs], ALU.mult)
                wa = gp.tile([128, 16], F32, tag="wa", name=f"wa{t}")
                nc.vector.tensor_tensor(wa[:], ua[:], d[:], ALU.mult)
                hT2 = hTp.tile([128, 16], F32, tag="hT", name=f"hT{t}")
                nc.vector.tensor_tensor(rr(hT2[:]), hT[:], wa[:], ALU.add)
            # h_new (unmasked) into bf16 history
            w_ = gp.tile([128, 16], F32, tag="w", name=f"w{t}")
            nc.gpsimd.tensor_tensor(w_[:], u[:], d[:], ALU.mult)
            nc.vector.tensor_tensor(hnewT[:, cs], hT[:], w_[:], ALU.add)
            if t < T - 1:
                hT = hT2

            if t % 16 == 15:
                attn_block(t // 16)

            if t >= 32:
                for k in range(2):
                    j = nchunk[0]
                    if j < NVCH:
                        p5_chunk(0, j, "v" if k == 0 else "s")
                        nchunk[0] = j + 1
                        if j % 2 == 1 and wo_next[0] < NJ2:
                            emit_wo(0, wo_next[0])
                            wo_next[0] += 1

        # ---- post-loop: m0 exps, m1 chunks (lts2 first), drains ----
        for j2 in range(3):
            emit_wo(1, j2)
        wo_next[1] = 3
        for g in range(NEGR):
            p5_exp(0, g)
        lse0 = nlse0 = None
        drain0 = 0
        for j in range(NVCH):
            p5_chunk(1, j, "v")
            if j % 2 == 1 and wo_next[1] < NJ2:
                emit_wo(1, wo_next[1])
                wo_next[1] += 1
            if j % 4 == 3:
                p5_exp(1, j // 4)
            if j == 12:
                lse0, nlse0 = p5_lse(0)
            # m1 writes into lts proper start at j=16 (lts2 covers j<16);
            # drain m0's chunk g before m1 overwrites that region.
            if lse0 is not None and drain0 < V // OC:
                want = min((j + 2) * VCHUNK // OC + 1, V // OC)
                while drain0 < want:
                    p5_drain_chunk(0, drain0, lse0, nlse0)
                    drain0 += 1
        while drain0 < V // OC:
            p5_drain_chunk(0, drain0, lse0, nlse0)
            drain0 += 1
        lse1, nlse1 = p5_lse(1)
        for g in range(V // OC):
            p5_drain_chunk(1, g, lse1, nlse1)

    nc.compile()
    return nc


_NC_CACHE = None


def _get_program():
    global _NC_CACHE
    if _NC_CACHE is None:
        _NC_CACHE = build_program()
    return _NC_CACHE


def make_core_inputs(all_encoder_hidden_states, initial_decoder_hidden_state,
                     encoder_output_mask, target_input, fra_length, embedding,
                     W_ih, W_hh, b_ih, b_hh, W_cc, b_cc, W_out, b_out):
    """Build the per-core input maps (host-side sharding/layout only)."""
    bf = ml_dtypes.bfloat16
    enc = np.ascontiguousarray(np.asarray(all_encoder_hidden_states, np.float32))
    h0 = np.asarray(initial_decoder_hidden_state, np.float32)[0]
    mask = np.asarray(encoder_output_mask)
    tgt = np.asarray(target_input).astype(np.int64)
    fra = np.asarray(fra_length).astype(np.int64)
    emb = np.ascontiguousarray(np.asarray(embedding, np.float32))
    wihT = np.ascontiguousarray(np.asarray(W_ih, np.float32).T.astype(bf))
    whhT = np.asarray(W_hh, np.float32).T.copy()
    whhT[:, 2 * H:] *= 0.5          # n-gate pre-scaled by 0.5
    b_ih = np.asarray(b_ih, np.float32)
    b_hh = np.asarray(b_hh, np.float32)
    bgx = np.concatenate([b_ih[:2 * H] + b_hh[:2 * H], b_ih[2 * H:]])[None, :]
    bhn2 = (0.5 * b_hh[2 * H:])[None, :]
    wccT = np.ascontiguousarray(np.asarray(W_cc, np.float32).T.astype(bf))
    bcc4 = np.ascontiguousarray(np.asarray(b_cc, np.float32).reshape(4, 128).T)
    woT = np.ascontiguousarray(np.asarray(W_out, np.float32).T.astype(bf))

    in_maps = []
    for c in range(NC):
        bs = slice(c * BL, (c + 1) * BL)
        enc_c = enc[bs]                                   # [BL, S, H]
        ids = tgt[bs].T.reshape(R).astype(np.int32)       # r = t*BL + b
        h0c = h0[bs]                                      # [BL, H]
        h0T = np.ascontiguousarray(
            h0c.reshape(BL, 4, 128).transpose(2, 1, 0).reshape(128, 16))
        in_maps.append({
            "emb": emb,
            "ids": np.ascontiguousarray(ids.reshape(2, 128, 1)),
            "h0T": h0T,
            "encT": np.ascontiguousarray(
                enc_c.transpose(2, 0, 1).reshape(H, BL * S).astype(bf)),
            "encS": np.ascontiguousarray(
                enc_c.transpose(1, 0, 2).reshape(S, BL * H).astype(bf)),
            "maskb": np.ascontiguousarray(
                np.where(mask[bs], 0.0, NEG).astype(bf).reshape(1, BL * S)),
            "actmT": np.ascontiguousarray(np.broadcast_to(
                np.tile(
                    (np.arange(T)[:, None] < fra[bs][None, :])
                    .astype(np.float32),
                    (1, 4),
                ).reshape(1, T * 16),
                (128, T * 16),
            )),
            "wihT": wihT,
            "whhT": np.ascontiguousarray(whhT),
            "bgx": np.ascontiguousarray(bgx),
            "bhn2": np.ascontiguousarray(bhn2),
            "wccT": wccT,
            "bcc": bcc4,
            "woT": woT,
            "onesd": np.ones((1, 256), np.float32),
        })
    return in_maps


def assemble_output(core_outs):
    """core_outs: list of 8 arrays [R, V] bf16 (rows r = t*BL + b)."""
    out = np.empty((B, T, V), np.float32)
    for c in range(NC):
        o = np.asarray(core_outs[c]).astype(np.float32).reshape(T, BL, V)
        out[c * BL:(c + 1) * BL] = o.transpose(1, 0, 2)
    return out


def kernel(**inputs) -> np.ndarray:
    from concourse.bass_utils import run_bass_kernel_spmd
    nc = _get_program()
    in_maps = make_core_inputs(**inputs)
    res = run_bass_kernel_spmd(nc, in_maps, list(range(NC)))
    return assemble_output([res.results[c]["out"] for c in range(NC)])


# revision 21
# speedup vs baseline: 1.0005x; 1.0005x over previous
"""Trainium2 Bass kernel for a GRU decoder with Luong attention.

Problem (hardcoded shapes): B=32, S=64, T=64, H=512, V=32000.
  out = log_softmax(decoder(inputs)) with shape [B, T, V] fp32.

Sharding: data-parallel over batch. Each of the 8 cores processes 4 batch
rows end-to-end. No collectives.

Layout: recurrence state is kept transposed, hT [128, 16] with
column = q*4 + b meaning h[b, q*128 + p] for partition p. Gate pre-acts
are computed as ghT = W_hh^T-block @ hT (weight-stationary, tiny moving
size), with the precomputed gxT folded into PSUM via an identity matmul.
Gates use tanh only (sigmoid(x) = 0.5 + 0.5*tanh(x/2)) so the whole body
runs on one activation table (tanh+exp).

Output phase: logits for rows t=0..31 (m0) are computed interleaved with
recurrence steps t=32..63; m1 afterwards. Logits are stashed in SBUF as
bf16, exp+accum gives the per-row sum, log-softmax subtract is a
tensor_scalar / biased-Identity, output tensor is bf16 (host casts).

Per-core row order for the 256 output rows is t-major: r = t*4 + b_local.
"""

from contextlib import ExitStack

import numpy as np
import ml_dtypes

import concourse.bacc as bacc
import concourse.bass as bass
import concourse.mybir as mybir
import concourse.tile as tile
from concourse.masks import make_identity

F32 = mybir.dt.float32
BF16 = mybir.dt.bfloat16
I32 = mybir.dt.int32
AF = mybir.ActivationFunctionType
ALU = mybir.AluOpType
AX = mybir.AxisListType
F32R = mybir.dt.float32r


def rr(ap):
    return ap.bitcast(F32R)


B, S, T, H, V = 32, 64, 64, 512, 32000
NC = 8
BL = B // NC          # 4 local batch rows
R = T * BL            # 256 local output rows, r = t*BL + b
VCHUNK = 500          # vocab chunk for the output matmul
NVCH = V // VCHUNK    # 64 chunks per m-block
NJ2 = NVCH // 2       # 32 wo-tile groups per m-block ([128,1000] tiles)
EGRP = 2000           # exp group width
NEGR = V // EGRP      # 16 exp groups per m-block
L2W = 8000            # lts2 width (m1 chunks j < 16 land here)
NEG = -1e30
OC = 2000             # drain/store chunk


def build_program():
    nc = bacc.Bacc(None, target_bir_lowering=False, debug=False)

    # ---- DRAM parameters (per-core slices prepared on host) ----
    emb_d = nc.declare_dram_parameter("emb", [V, H], F32, isOutput=False)
    ids_d = nc.declare_dram_parameter("ids", [2, 128, 1], I32, isOutput=False)
    h0T_d = nc.declare_dram_parameter("h0T", [128, 16], F32, isOutput=False)
    encT_d = nc.declare_dram_parameter("encT", [H, BL * S], BF16, isOutput=False)
    encS_d = nc.declare_dram_parameter("encS", [S, BL * H], BF16, isOutput=False)
    maskb_d = nc.declare_dram_parameter("maskb", [1, BL * S], BF16, isOutput=False)
    actmT_d = nc.declare_dram_parameter("actmT", [128, T * 16], F32, isOutput=False)
    wihT_d = nc.declare_dram_parameter("wihT", [H, 3 * H], BF16, isOutput=False)
    whhT_d = nc.declare_dram_parameter("whhT", [H, 3 * H], F32, isOutput=False)
    bgx_d = nc.declare_dram_parameter("bgx", [1, 3 * H], F32, isOutput=False)
    bhn2_d = nc.declare_dram_parameter("bhn2", [1, H], F32, isOutput=False)
    wccT_d = nc.declare_dram_parameter("wccT", [2 * H, H], BF16, isOutput=False)
    bcc_d = nc.declare_dram_parameter("bcc", [128, 4], F32, isOutput=False)
    woT_d = nc.declare_dram_parameter("woT", [H, V], BF16, isOutput=False)
    ones_d = nc.declare_dram_parameter("onesd", [1, 256], F32, isOutput=False)
    out_d = nc.declare_dram_parameter("out", [R, V], BF16, isOutput=True)

    with tile.TileContext(nc) as tc, ExitStack() as stk:
        constp = stk.enter_context(tc.tile_pool(name="const", bufs=1))
        histp = stk.enter_context(tc.tile_pool(name="hist", bufs=1))
        hotp = stk.enter_context(tc.tile_pool(name="hot", bufs=1))
        ltsp = stk.enter_context(tc.tile_pool(name="lts", bufs=1))
        woPp = stk.enter_context(tc.tile_pool(name="woP", bufs=3))
        woSp = stk.enter_context(tc.tile_pool(name="woS", bufs=3))
        edp = stk.enter_context(tc.tile_pool(name="edump", bufs=1))
        sump = stk.enter_context(tc.tile_pool(name="sums", bufs=6))
        osp = stk.enter_context(tc.tile_pool(name="ost", bufs=2))
        hTp = stk.enter_context(tc.tile_pool(name="hT", bufs=3))
        gp = stk.enter_context(tc.tile_pool(name="gates", bufs=3))
        attp = stk.enter_context(tc.tile_pool(name="att", bufs=2))
        ps_g = stk.enter_context(tc.tile_pool(name="ps_g", bufs=3, space="PSUM"))
        ps_a = stk.enter_context(tc.tile_pool(name="ps_a", bufs=2, space="PSUM"))
        ps_o = stk.enter_context(tc.tile_pool(name="ps_o", bufs=3, space="PSUM"))

        # ---- constants / weights ----
        ident = constp.tile([128, 128], F32, tag="ident")
        make_identity(nc, ident[:])

        ones_f = constp.tile([1, 256], F32, tag="ones_f")
        nc.sync.dma_start(rr(ones_f[:]), rr(ones_d[:]))
        ones_b = constp.tile([1, 128], BF16, tag="ones_b")
        nc.vector.memset(ones_b[:], 1.0)

        half = constp.tile([128, 1], F32, tag="half")
        nc.vector.memset(half[:], 0.5)
        h0T = constp.tile([128, 16], F32, tag="h0T")
        nc.sync.dma_start(rr(h0T[:]), rr(h0T_d[:]))
        identr = constp.tile([128, 128], F32, tag="identr")
        bhn2 = constp.tile([1, H], F32, tag="bhn2")
        nc.sync.dma_start(rr(bhn2[:]), rr(bhn2_d[:]))
        whh = []
        for q in range(4):
            ht = constp.tile([128, 3 * H], F32, tag=f"whh{q}")
            eng = nc.scalar if q % 2 == 0 else nc.sync
            eng.dma_start(rr(ht[:]), rr(whhT_d[q * 128:(q + 1) * 128, :]))
            whh.append(ht)

        # embedding gather first on the Pool queue (P1 critical path)
        gat = []
        for m in range(2):
            ids_t = constp.tile([128, 1], I32, tag=f"ids{m}", name=f"ids{m}")
            nc.gpsimd.dma_start(ids_t[:], ids_d[m])
            xs_t = constp.tile([128, H], F32, tag=f"xs{m}", name=f"xs{m}")
            nc.gpsimd.indirect_dma_start(
                out=xs_t[:],
                out_offset=None,
                in_=emb_d[:],
                in_offset=bass.IndirectOffsetOnAxis(ap=ids_t[:, 0:1], axis=0),
            )
            gat.append(xs_t)
        maskb = constp.tile([1, BL * S], BF16, tag="maskb")
        nc.gpsimd.dma_start(maskb[:], maskb_d[:])
        actmT = constp.tile([128, T * 16], F32, tag="actmT")
        nc.gpsimd.dma_start(actmT[:], actmT_d[:])
        bcc = constp.tile([128, 4], F32, tag="bcc")
        nc.gpsimd.dma_start(bcc[:], bcc_d[:])
        encT = []
        for q in range(4):
            e = constp.tile([128, BL * S], BF16, tag=f"encT{q}")
            nc.gpsimd.dma_start(e[:], encT_d[q * 128:(q + 1) * 128, :])
            encT.append(e)
        encS = constp.tile([S, BL * H], BF16, tag="encS")
        nc.gpsimd.dma_start(encS[:], encS_d[:])
        wcc = []
        for kt in range(8):
            w = constp.tile([128, H], BF16, tag=f"wcc{kt}")
            nc.gpsimd.dma_start(w[:], wccT_d[kt * 128:(kt + 1) * 128, :])
            wcc.append(w)

        # history buffers: col = t*16 + q*4 + b
        nc.vector.tensor_copy(rr(identr[:]), ident[:])
        hnewT = histp.tile([128, T * 16], BF16, tag="hnewT")
        ctxT = histp.tile([128, T * 16], BF16, tag="ctxT")
        gxT = [histp.tile([128, T * 16], F32, tag=f"gxT{g}", name=f"gxT{g}")
           for g in range(3)]
        hot = [hotp.tile([128, R], BF16, tag=f"hot{q}", name=f"hot{q}")
           for q in range(4)]
        lts = ltsp.tile([128, V], BF16, tag="lts")
        lts2 = ltsp.tile([128, L2W], BF16, tag="lts2")
        sets = [sump.tile([128, NEGR], F32, tag=f"se{m}", name=f"sums{m}")
                for m in range(2)]

        def lts_ap(m, c0, c1):
            if m == 1 and c1 <= L2W:
                return lts2[:, c0:c1]
            return lts[:, c0:c1]

        # ---- P1: embedding gather + gxT = (xs @ W_ih.T + bias)^T ----
        with tc.tile_pool(name="p1", bufs=1) as p1p, \
             tc.tile_pool(name="p1x", bufs=2) as p1xp:
            bgx = p1p.tile([1, 3 * H], F32, tag="bgx")
            nc.sync.dma_start(rr(bgx[:]), rr(bgx_d[:]))
            wih = []
            for q in range(4):
                wt = p1p.tile([128, 3 * H], BF16, tag=f"wih{q}")
                nc.sync.dma_start(wt[:], wihT_d[q * 128:(q + 1) * 128, :])
                wih.append(wt)
            xsT = []
            for q in range(4):
                xt = p1p.tile([128, 256], BF16, tag=f"xsT{q}")
                xsT.append(xt)
            for m in range(2):
                xs_t = gat[m]
                for q in range(4):
                    tp = ps_a.tile([128, 128], F32, tag="A", name=f"tp{m}_{q}")
                    nc.tensor.transpose(tp[:], xs_t[:, q * 128:(q + 1) * 128],
                                        ident[:])
                    nc.vector.tensor_copy(xsT[q][:, m * 128:(m + 1) * 128],
                                          tp[:])
            for g in range(3):
                for qo in range(4):
                    c = g * 512 + qo * 128
                    ps = ps_a.tile([128, 256], F32, tag="A", name=f"gx{g}_{qo}")
                    for qi in range(4):
                        nc.tensor.matmul(
                            ps[:], wih[qi][:, c:c + 128], xsT[qi][:],
                            start=(qi == 0), stop=False,
                        )
                    nc.tensor.matmul(
                        ps[:], rr(bgx[0:1, c:c + 128]), rr(ones_f[0:1, :]),
                        start=False, stop=True,
                    )
                    dst = rr(gxT[g][:]).rearrange("p (t s) -> p t s", s=16)[
                        :, :, qo * 4:(qo + 1) * 4]
                    src = ps[:].rearrange("p (t b) -> p t b", b=4)
                    nc.vector.tensor_copy(dst, src)

        # ---- W_out prefetch tiles: [128, 1000] bf16, keyed (m, j2, q) ----
        # Pool queue carries q in {0,1}; SP queue carries q in {2,3}.
        wo_tiles = {}

        def emit_wo(m, j2):
            for q in range(4):
                pool = woPp if q < 2 else woSp
                eng = nc.gpsimd if q < 2 else nc.sync
                w = pool.tile([128, 1000], BF16, tag=f"wo{q}",
                              name=f"wo{m}_{j2}_{q}")
                eng.dma_start(
                    w[:], woT_d[q * 128:(q + 1) * 128,
                                j2 * 1000:(j2 + 1) * 1000])
                wo_tiles[(m, j2, q)] = w

        # ---- P5 chunk: logits for rows m*128..+128, vocab j*500..+500 ----
        def p5_chunk(m, j, cp_eng):
            j2, half = j // 2, j % 2
            hs = slice(half * VCHUNK, (half + 1) * VCHUNK)
            ps = ps_o.tile([128, VCHUNK], F32, tag="O", name=f"po{m}_{j}")
            for q in range(4):
                wt = wo_tiles[(m, j2, q)]
                if half == 1 and q == 3:
                    del wo_tiles[(m, j2, q)]
                nc.tensor.matmul(
                    ps[:], hot[q][:, m * 128:(m + 1) * 128], wt[:, hs],
                    start=(q == 0), stop=(q == 3),
                )
            dst = lts_ap(m, j * VCHUNK, (j + 1) * VCHUNK)
            if cp_eng == "v":
                nc.vector.tensor_copy(dst, ps[:])
            else:
                nc.scalar.activation(dst, ps[:], AF.Identity)

        # ---- P5 exp group: sum of exp over a 2000-wide slice ----
        def p5_exp(m, g):
            dump = edp.tile([128, EGRP], BF16, tag="edump", name=f"ed{m}_{g}")
            nc.scalar.activation(
                dump[:], lts_ap(m, g * EGRP, (g + 1) * EGRP), AF.Exp,
                accum_out=sets[m][:, g:g + 1],
            )

        # ---- P5 drain: lse + subtract + store (descending if asked) ----
        def p5_lse(m):
            stot = sump.tile([128, 1], F32, tag="stot", name=f"st{m}")
            scr = sump.tile([128, NEGR], F32, tag="scr", name=f"scr{m}")
            nc.scalar.activation(scr[:], sets[m][:], AF.Identity,
                                 accum_out=stot[:, 0:1])
            lse = sump.tile([128, 1], F32, tag="lse", name=f"ls{m}")
            nc.scalar.activation(lse[:], stot[:], AF.Ln)
            nlse = sump.tile([128, 1], F32, tag="nlse", name=f"nls{m}")
            nc.vector.tensor_scalar_mul(nlse[:], lse[:], -1.0)
            return lse, nlse

        def p5_drain_chunk(m, g, lse, nlse):
            ost = osp.tile([128, OC], BF16, tag="ost", name=f"os{m}_{g}")
            src = lts_ap(m, g * OC, (g + 1) * OC)
            if g % 4 < 3:
                nc.vector.tensor_scalar_sub(ost[:], src, lse[:, 0:1])
            else:
                nc.scalar.activation(ost[:], src, AF.Identity,
                                     bias=nlse[:, 0:1])
            nc.sync.dma_start(
                out_d[m * 128:(m + 1) * 128, g * OC:(g + 1) * OC], ost[:])

        # ---- attention + hot for a 16-step block ----
        def attn_block(blk):
            c0, c1 = blk * 256, (blk + 1) * 256
            for b in range(BL):
                sc = ps_a.tile([16, S], F32, tag="A", name=f"sc{blk}_{b}")
                for q in range(4):
                    nc.tensor.matmul(
                        sc[:], hnewT[:, c0 + q * 4 + b:c1:16],
                        encT[q][:, b * S:(b + 1) * S],
                        start=(q == 0), stop=False,
                    )
                nc.tensor.matmul(
                    sc[:], ones_b[0:1, 0:16], maskb[0:1, b * S:(b + 1) * S],
                    start=False, stop=True,
                )
                se = attp.tile([16, 1], F32, tag="se", name=f"se{blk}_{b}")
                al = attp.tile([16, S], F32, tag="al", name=f"al{blk}_{b}")
                nc.scalar.activation(al[:], sc[:], AF.Exp,
                                     accum_out=se[:, 0:1])
                rec = attp.tile([16, 1], F32, tag="rec", name=f"rc{blk}_{b}")
                nc.vector.reciprocal(rec[:], se[:])
                aln = attp.tile([16, S], F32, tag="aln", name=f"an{blk}_{b}")
                nc.vector.tensor_scalar_mul(aln[:], al[:], rec[:, 0:1])
                alT_ps = ps_a.tile([S, 16], F32, tag="A", name=f"tp{blk}_{b}")
                nc.tensor.transpose(alT_ps[:], aln[:], ident[0:16, 0:16])
                alT = attp.tile([S, 16], BF16, tag="alT", name=f"at{blk}_{b}")
                nc.vector.tensor_copy(alT[:], alT_ps[:])
                for q in range(4):
                    cx = ps_a.tile([128, 16], F32, tag="A",
                                   name=f"cx{blk}_{b}_{q}")
                    nc.tensor.matmul(
                        cx[:],
                        encS[0:S, b * H + q * 128:b * H + (q + 1) * 128],
                        alT[:],
                        start=True, stop=True,
                    )
                    nc.vector.tensor_copy(ctxT[:, c0 + q * 4 + b:c1:16], cx[:])
            for mh in range(4):
                hps = ps_a.tile([128, 64], F32, tag="A", name=f"hp{blk}_{mh}")
                for kt in range(8):
                    srcT = ctxT if kt < 4 else hnewT
                    q = kt % 4
                    rhs = srcT[:].rearrange("p (t x) -> p t x", x=16)[
                        :, blk * 16:(blk + 1) * 16, q * 4:(q + 1) * 4]
                    nc.tensor.matmul(
                        hps[:], wcc[kt][:, mh * 128:(mh + 1) * 128], rhs,
                        start=(kt == 0), stop=(kt == 7),
                    )
                nc.scalar.activation(
                    hot[mh][:, blk * 64:(blk + 1) * 64], hps[:],
                    AF.Tanh, bias=bcc[:, mh:mh + 1],
                )

        # ---- P2: GRU recurrence over T steps, P5-m0 woven in ----
        hT = h0T
        for j2 in range(3):
            emit_wo(0, j2)
        wo_next = [3, 0]
        nchunk = [0, 0]

        for t in range(T):
            cs = slice(t * 16, (t + 1) * 16)
            psums = [None, None, None]
            for g in (0, 1, 2):
                pg = ps_g.tile([128, 16], F32, tag="G", name=f"pg{t}_{g}")
                for qo in range(4):
                    co = slice(qo * 4, (qo + 1) * 4)
                    if g < 2:
                        nc.tensor.matmul(
                            pg[:, co], rr(identr[:]),
                            rr(gxT[g][:, t * 16 + qo * 4:t * 16 + qo * 4 + 4]),
                            start=True, stop=False,
                        )
                    else:
                        nc.tensor.matmul(
                            pg[:, co],
                            rr(bhn2[0:1, qo * 128:(qo + 1) * 128]),
                            rr(ones_f[0:1, 0:4]),
                            start=True, stop=False,
                        )
                    c = g * 512 + qo * 128
                    for qi in range(4):
                        nc.tensor.matmul(
                            pg[:, co],
                            rr(whh[qi][:, c:c + 128]),
                            rr(hT[:, qi * 4:(qi + 1) * 4]),
                            start=False, stop=(qi == 3),
                        )
                psums[g] = pg
            pr, pz, pn = psums
            # r = 0.5*(1+tanh(0.5*pre)) ; u = 1-z = 0.5*(1+tanh(-0.5*pre))
            th_r = gp.tile([128, 16], F32, tag="th_r", name=f"thr{t}")
            nc.scalar.activation(th_r[:], pr[:], AF.Tanh, scale=0.5)
            th_u = gp.tile([128, 16], F32, tag="th_u", name=f"thu{t}")
            nc.scalar.activation(th_u[:], pz[:], AF.Tanh, scale=-0.5)
            # n path: pn = 0.5*(gh_n + b_hn);  r*(gh_n + b_hn) = (1+th_r)*pn
            t1 = gp.tile([128, 16], F32, tag="t1", name=f"t1_{t}")
            nc.vector.tensor_tensor(t1[:], th_r[:], pn[:], ALU.mult)
            t2 = gp.tile([128, 16], F32, tag="t2", name=f"t2_{t}")
            nc.vector.tensor_tensor(t2[:], pn[:], gxT[2][:, cs], ALU.add)
            npre = gp.tile([128, 16], F32, tag="npre", name=f"np{t}")
            nc.vector.tensor_tensor(npre[:], t1[:], t2[:], ALU.add)
            nT = gp.tile([128, 16], F32, tag="nT", name=f"nT{t}")
            nc.scalar.activation(nT[:], npre[:], AF.Tanh)
            u = gp.tile([128, 16], F32, tag="u", name=f"u{t}")
            nc.scalar.activation(u[:], th_u[:], AF.Identity,
                                 bias=half[:, 0:1], scale=0.5)
            d = gp.tile([128, 16], F32, tag="d", name=f"d{t}")
            nc.vector.tensor_tensor(d[:], nT[:], hT[:], ALU.subtract)
            if t < T - 1:
                ua = gp.tile([128, 16], F32, tag="ua", name=f"ua{t}")
                nc.vector.tensor_tensor(ua[:], u[:], actmT[:, cs], ALU.mult)
                wa = gp.tile([128, 16], F32, tag="wa", name=f"wa{t}")
                nc.vector.tensor_tensor(wa[:], ua[:], d[:], ALU.mult)
                hT2 = hTp.tile([128, 16], F32, tag="hT", name=f"hT{t}")
                nc.vector.tensor_tensor(rr(hT2[:]), hT[:], wa[:], ALU.add)
            # h_new (unmasked) into bf16 history
            w_ = gp.tile([128, 16], F32, tag="w", name=f"w{t}")
            nc.gpsimd.tensor_tensor(w_[:], u[:], d[:], ALU.mult)
            nc.vector.tensor_tensor(hnewT[:, cs], hT[:], w_[:], ALU.add)
            if t < T - 1:
                hT = hT2

            if t % 16 == 15:
                attn_block(t // 16)

            if t >= 32:
                for k in range(2):
                    j = nchunk[0]
                    if j < NVCH:
                        p5_chunk(0, j, "v" if k == 0 else "s")
                        nchunk[0] = j + 1
                        if j % 2 == 1 and wo_next[0] < NJ2:
                            emit_wo(0, wo_next[0])
                            wo_next[0] += 1

        # ---- post-loop: m0 exps, m1 chunks (lts2 first), drains ----
        for j2 in range(3):
            emit_wo(1, j2)
        wo_next[1] = 3
        for g in range(NEGR):
            p5_exp(0, g)
        lse0 = nlse0 = None
        drain0 = 0
        for j in range(NVCH):
            p5_chunk(1, j, "v")
            if j % 2 == 1 and wo_next[1] < NJ2:
                emit_wo(1, wo_next[1])
                wo_next[1] += 1
            if j % 4 == 3:
                p5_exp(1, j // 4)
            if j == 12:
                lse0, nlse0 = p5_lse(0)
            # m1 writes into lts proper start at j=16 (lts2 covers j<16);
            # drain m0's chunk g before m1 overwrites that region.
            if lse0 is not None and drain0 < V // OC:
                want = min((j + 2) * VCHUNK // OC + 1, V // OC)
                while drain0 < want:
                    p5_drain_chunk(0, drain0, lse0, nlse0)
                    drain0 += 1
        while drain0 < V // OC:
            p5_drain_chunk(0, drain0, lse0, nlse0)
            drain0 += 1
        lse1, nlse1 = p5_lse(1)
        for g in range(V // OC):
            p5_drain_chunk(1, g, lse1, nlse1)

    nc.compile()
    return nc


_NC_CACHE = None


def _get_program():
    global _NC_CACHE
    if _NC_CACHE is None:
        _NC_CACHE = build_program()
    return _NC_CACHE


def make_core_inputs(all_encoder_hidden_states, initial_decoder_hidden_state,
                     encoder_output_mask, target_input, fra_length, embedding,
                     W_ih, W_hh, b_ih, b_hh, W_cc, b_cc, W_out, b_out):
    """Build the per-core input maps (host-side sharding/layout only)."""
    bf = ml_dtypes.bfloat16
    enc = np.ascontiguousarray(np.asarray(all_encoder_hidden_states, np.float32))
    h0 = np.asarray(initial_decoder_hidden_state, np.float32)[0]
    mask = np.asarray(encoder_output_mask)
    tgt = np.asarray(target_input).astype(np.int64)
    fra = np.asarray(fra_length).astype(np.int64)
    emb = np.ascontiguousarray(np.asarray(embedding, np.float32))
    wihT = np.ascontiguousarray(np.asarray(W_ih, np.float32).T.astype(bf))
    whhT = np.asarray(W_hh, np.float32).T.copy()
    whhT[:, 2 * H:] *= 0.5          # n-gate pre-scaled by 0.5
    b_ih = np.asarray(b_ih, np.float32)
    b_hh = np.asarray(b_hh, np.float32)
    bgx = np.concatenate([b_ih[:2 * H] + b_hh[:2 * H], b_ih[2 * H:]])[None, :]
    bhn2 = (0.5 * b_hh[2 * H:])[None, :]
    wccT = np.ascontiguousarray(np.asarray(W_cc, np.float32).T.astype(bf))
    bcc4 = np.ascontiguousarray(np.asarray(b_cc, np.float32).reshape(4, 128).T)
    woT = np.ascontiguousarray(np.asarray(W_out, np.float32).T.astype(bf))

    in_maps = []
    for c in range(NC):
        bs = slice(c * BL, (c + 1) * BL)
        enc_c = enc[bs]                                   # [BL, S, H]
        ids = tgt[bs].T.reshape(R).astype(np.int32)       # r = t*BL + b
        h0c = h0[bs]                                      # [BL, H]
        h0T = np.ascontiguousarray(
            h0c.reshape(BL, 4, 128).transpose(2, 1, 0).reshape(128, 16))
        in_maps.append({
            "emb": emb,
            "ids": np.ascontiguousarray(ids.reshape(2, 128, 1)),
            "h0T": h0T,
            "encT": np.ascontiguousarray(
                enc_c.transpose(2, 0, 1).reshape(H, BL * S).astype(bf)),
            "encS": np.ascontiguousarray(
                enc_c.transpose(1, 0, 2).reshape(S, BL * H).astype(bf)),
            "maskb": np.ascontiguousarray(
                np.where(mask[bs], 0.0, NEG).astype(bf).reshape(1, BL * S)),
            "actmT": np.ascontiguousarray(np.broadcast_to(
                np.tile(
                    (np.arange(T)[:, None] < fra[bs][None, :])
                    .astype(np.float32),
                    (1, 4),
                ).reshape(1, T * 16),
                (128, T * 16),
            )),
            "wihT": wihT,
            "whhT": np.ascontiguousarray(whhT),
            "bgx": np.ascontiguousarray(bgx),
            "bhn2": np.ascontiguousarray(bhn2),
            "wccT": wccT,
            "bcc": bcc4,
            "woT": woT,
            "onesd": np.ones((1, 256), np.float32),
        })
    return in_maps


def assemble_output(core_outs):
    """core_outs: list of 8 arrays [R, V] bf16 (rows r = t*BL + b)."""
    out = np.empty((B, T, V), np.float32)
    for c in range(NC):
        o = np.asarray(core_outs[c]).astype(np.float32).reshape(T, BL, V)
        out[c * BL:(c + 1) * BL] = o.transpose(1, 0, 2)
    return out


def kernel(**inputs) -> np.ndarray:
    from concourse.bass_utils import run_bass_kernel_spmd
    nc = _get_program()
    in_maps = make_core_inputs(**inputs)
    res = run_bass_kernel_spmd(nc, in_maps, list(range(NC)))
    return assemble_output([res.results[c]["out"] for c in range(NC)])
